# revision 8
# baseline (speedup 1.0000x reference)
"""Trainium2 Bass kernel for nn_ConditionalPoolingModule (v4).

Reference computation (per scene s of 64, peds i,j of 64):
    feat[s,i,j]  = [pos_j - pos_i, speed_j]
    emb          = feat @ W_emb + b_emb
    x1[s,i,j]    = relu(bn1(concat(h_j, emb) @ W1 + b1))      # [.., 512]
    x2[s,i,j]    = relu(bn2(x1 @ W2 + b2))                    # [.., 256]
    out[s,i]     = max_j x2[s,i,j]

Algebra (same as v1):
  * Layer 1 separable: bn1-affine = A''[j] - B''[i], A'' from X=[h,pos,spd]@W1aug,
    B'' rank-2 from pos. relu(a-b) = max(a,b)-b commutes with layer 2 + j-max:
      out[i] = relu( max_j( max(A''[j], B''[i]) @ W2s ) - B''[i]@W2s + t2 )

Pipeline structure (per core: 8 scenes, steady state is PE/DVE co-critical at
~14us/scene; everything else hides under it):
  * PE: per scene 16 blocks x 4 fp16 matmuls (contraction 512 over 4 chunks)
    + 8 tiny D matmuls. ACT: 16 PSUM->SBUF fp16 copies + negD + relu.
    DVE: 4 mx TTs (max(A_j, B_i), 2x_1p) + 2 whole-scene j-max TT trees.
  * Epilogue entirely in [b, i] layout: negD = t2 - B''@W2s via fp16 matmuls
    on the q=0 lane of B_dup (the same fp16 values the mx path sees), t2 as
    per-partition ACT bias. No transposes; host transposes the output once.
  * tree(s-1) emitted after mx(s) so DVE tree latency never starves PE.

Start/tail engineering:
  * All constants packed host-side into 4 input DMAs, ordered by need
    (HWDGE serializes DMAs at ~625ns each, so fewer, larger DMAs start
    compute sooner). A dummy activation on a memset tile triggers the
    1.28us ACT table load at t~0.2us, off the critical chain.
  * Phase-0 emitted in column pieces (scenes 0-1 first), scene-0 mx in
    (16, 48)-i pieces: the first layer-2 matmul issues at ~7us, not ~12us.
  * Last scene: tree pieces [0:32, 32:48, 48:64] in copy-arrival order
    (m-interleaved) with per-piece add/relu/DMA, so the tail after the
    last matmul is one small tree piece + one small DMA. Piece bounds are
    tuned: finer splits cost more DVE init overhead than they save in
    tail latency.

Measured (TimelineSim cost model, the grading metric): 126954 ns vs the
143508 ns v1 baseline; relative error 8.8e-4 vs the fp32 reference
(tolerance 2e-2). PE busy 112.7us (layer-2 floor 109.2us at fp16
1 cyc/row), DVE busy ~114.7us - both ~90% occupied; the remainder is DMA
start latency, the final DMA+barrier chain, and pipeline-fill.
"""
import numpy as np

import concourse.bacc as bacc
import concourse.tile as tile
from concourse import mybir
from concourse.bass_utils import run_bass_kernel_spmd

EPS = 1e-5
S, P = 64, 64
H, E = 64, 16
MID, BOT = 512, 256
KIN = H + 3            # 67: h(64) + posx + posy + speed
NCORES = 8
S_LOC = S // NCORES    # 8 scenes per core
NLOC = S_LOC * P       # 512 peds per core
KC = MID // 128        # 4 mid chunks
MC = BOT // 128        # 2 bot chunks
F32 = mybir.dt.float32
BF16 = mybir.dt.float16  # fp16: values are O(1) here

_CACHE = {}


def _build_program():
    nc = bacc.Bacc("TRN2", target_bir_lowering=False, debug=False, num_devices=NCORES)

    # packed inputs (4 DMAs): see _prep_inputs for layouts
    big67 = nc.dram_tensor("big67", [KIN, 2 * NLOC], BF16, kind="ExternalInput").ap()
    pw1b = nc.dram_tensor("pw1b", [2, 2 * MID], BF16, kind="ExternalInput").ap()
    mvt2 = nc.dram_tensor("mvt2", [128, 2 * KC + MC], F32, kind="ExternalInput").ap()
    w2pack = nc.dram_tensor("w2pack", [128, 2 * KC * BOT], BF16,
                            kind="ExternalInput").ap()
    out_d = nc.dram_tensor("out", [BOT, NLOC], F32, kind="ExternalOutput").ap()

    with tile.TileContext(nc) as tc, \
         tc.tile_pool(name="const", bufs=1) as cpool, \
         tc.tile_pool(name="ab", bufs=1) as abpool, \
         tc.tile_pool(name="mx", bufs=2) as mxpool, \
         tc.tile_pool(name="y2", bufs=3) as ypool, \
         tc.tile_pool(name="tr", bufs=2) as tpool, \
         tc.tile_pool(name="work", bufs=2) as wpool, \
         tc.tile_pool(name="mm", bufs=5, space="PSUM") as mmpool, \
         tc.tile_pool(name="p0", bufs=1, space="PSUM") as p0pool, \
         tc.tile_pool(name="dps", bufs=1, space="PSUM") as dpool:

        # dummy act on a memset tile: pulls the ACT table load off the
        # critical chain (it runs during the input DMAs)
        warm = cpool.tile([1, 2], F32)
        nc.vector.memset(warm[:], 0.0)
        warm2 = cpool.tile([1, 2], F32, tag="warm2")
        nc.scalar.activation(warm2[:], warm[:],
                             mybir.ActivationFunctionType.Identity,
                             bias=0.0, scale=1.0)

        # ---- packed constant loads ----
        xw_sb = cpool.tile([KIN, 2 * NLOC], BF16)     # xaugT | w1augT
        pw_sb = cpool.tile([2, 2 * MID], BF16)        # posT | w1bT
        mv_all = cpool.tile([128, 2 * KC + MC], F32)  # mv chunks | t2 cols
        w2_sb = cpool.tile([128, 2 * KC * BOT], BF16)  # w2b chunks | w2n chunks
        nc.sync.dma_start(xw_sb[:], big67)
        nc.sync.dma_start(mv_all[:], mvt2)
        nc.sync.dma_start(pw_sb[:], pw1b)
        nc.sync.dma_start(w2_sb[:], w2pack)

        xaug_sb = xw_sb[:, 0:NLOC]
        w1aug_sb = xw_sb[:, NLOC:2 * NLOC]
        posT_sb = pw_sb[:, 0:MID]
        w1b_sb = pw_sb[:, MID:2 * MID]
        mv_sb = [mv_all[:, 2 * k:2 * k + 2] for k in range(KC)]
        t2_sb = mv_all[:, 2 * KC:2 * KC + MC]
        w2b_sb = [w2_sb[:, k * BOT:(k + 1) * BOT] for k in range(KC)]
        w2n_sb = [w2_sb[:, (KC + k) * BOT:(KC + k + 1) * BOT] for k in range(KC)]

        # ---- phase 0: A'' (fp16) and duplicated B'' (fp16) ----
        # A''[c, n] = s1*(X[n] @ W1aug)[c] + ca[c]
        # B_dup[c, 2n+q] = s1*(pos[n] @ R[:2])[c], q in {0, 1}
        # Emitted in column pieces so scene 0/1's slices are ready early.
        A_bf = [abpool.tile([128, NLOC], BF16, tag=f"A{k}", name=f"A_{k}")
                for k in range(KC)]
        B_dup = [abpool.tile([128, 2 * NLOC], BF16, tag=f"Bd{k}", name=f"Bd_{k}")
                 for k in range(KC)]

        def emit_phase0_piece(c0, c1):
            nh = slice(c0, c1)
            nh2 = slice(2 * c0, 2 * c1)
            for k in range(KC):
                ck = slice(k * 128, (k + 1) * 128)
                psA = p0pool.tile([128, c1 - c0], F32, tag=f"p0_{c1 - c0}",
                                  name=f"p0a_{c1 - c0}")
                nc.tensor.matmul(psA[:], lhsT=w1aug_sb[:, ck],
                                 rhs=xaug_sb[:, nh], start=True, stop=True)
                nc.scalar.activation(
                    A_bf[k][:, nh], psA[:],
                    mybir.ActivationFunctionType.Identity,
                    bias=mv_sb[k][:, 1:2], scale=mv_sb[k][:, 0:1])
                psB = p0pool.tile([128, c1 - c0], F32, tag=f"p0_{c1 - c0}",
                                  name=f"p0b_{c1 - c0}")
                nc.tensor.matmul(psB[:], lhsT=w1b_sb[:, ck],
                                 rhs=posT_sb[:, nh], start=True, stop=True)
                nc.scalar.activation(
                    B_dup[k][:, nh2].rearrange("c (n q) -> c n q", q=2),
                    psB[:].unsqueeze(2).broadcast_to((128, c1 - c0, 2)),
                    mybir.ActivationFunctionType.Identity,
                    bias=0.0, scale=mv_sb[k][:, 0:1])

        emit_phase0_piece(0, 128)

        # ---- per-scene pipeline ----
        def emit_tree(s, quarters, piece_epi=False):
            """j-max of y2(s) + add negD(s) + relu + DMA out. quarters=1 is
            one whole-scene tree per m (cheapest on DVE); quarters>1 splits
            by i-ranges so the tail after the last L2 block is short.
            piece_epi: add/relu/DMA per piece (last scene) so output DMAs
            overlap the remaining tree work."""
            y2, negD = scene_y2[s], scene_negD[s]
            if piece_epi:
                # pieces ordered to match ACT copy production (i-major),
                # m-inner so neither m blocks the other; the final quarter
                # is split into eighths so the last relu/DMA fire ASAP
                bounds = [0, 32, 48, 64]
            else:
                bounds = [qt * (P // quarters) for qt in range(quarters)] + [P]
            mrs = [wpool.tile([128, P], BF16, tag=f"mr{m}", name=f"mr_{m}")
                   for m in range(MC)]
            for i0, i1 in zip(bounds, bounds[1:]):
                ni = i1 - i0
                iq = slice(i0, i1)
                for m in range(MC):
                    mr = mrs[m]
                    y3 = y2[m][:, i0 * P:i1 * P].rearrange(
                        "c (i j) -> c i j", j=P)
                    cur = y3
                    width = P
                    while width > 1:
                        wh = width // 2
                        if width > 2:
                            nxt_t = tpool.tile([128, ni * wh], BF16,
                                               tag=f"r{m}_{wh}_{ni}",
                                               name=f"r{m}_{wh}_{ni}")
                            nxt = nxt_t[:].rearrange("c (i j) -> c i j", j=wh)
                        else:
                            nxt = mr[:, iq].unsqueeze(2)
                        nc.vector.tensor_tensor(
                            out=nxt, in0=cur[:, :, 0:wh], in1=cur[:, :, wh:width],
                            op=mybir.AluOpType.max)
                        cur = nxt
                        width = wh
                    if piece_epi:
                        sub_t = wpool.tile([128, P], BF16, tag=f"sub{m}",
                                           name=f"sub_{m}")
                        nc.vector.tensor_tensor(
                            out=sub_t[:, iq], in0=mr[:, iq], in1=negD[m][:, iq],
                            op=mybir.AluOpType.add)
                        o_sb = wpool.tile([128, P], F32, tag=f"osb{m}",
                                          name=f"osb_{m}")
                        nc.scalar.activation(o_sb[:, iq], sub_t[:, iq],
                                             mybir.ActivationFunctionType.Relu)
                        nc.sync.dma_start(
                            out_d[m * 128:(m + 1) * 128,
                                  s * P + i0:s * P + i1],
                            o_sb[:, iq])
            if not piece_epi:
                for m in range(MC):
                    sub_t = wpool.tile([128, P], BF16, tag=f"sub{m}",
                                       name=f"sub_{m}")
                    nc.vector.tensor_tensor(out=sub_t[:], in0=mrs[m][:],
                                            in1=negD[m][:],
                                            op=mybir.AluOpType.add)
                    o_sb = wpool.tile([128, P], F32, tag=f"osb{m}",
                                      name=f"osb_{m}")
                    nc.scalar.activation(o_sb[:], sub_t[:],
                                         mybir.ActivationFunctionType.Relu)
                    nc.sync.dma_start(
                        out_d[m * 128:(m + 1) * 128, s * P:(s + 1) * P], o_sb[:])

        scene_y2 = [None] * S_LOC
        scene_negD = [None] * S_LOC
        for s in range(S_LOC):
            cs = slice(s * P, (s + 1) * P)
            cs2 = slice(2 * s * P, 2 * (s + 1) * P)

            # D(s): negD[m][c, i] = -(B''_i @ W2s)[m*128+c] + t2[m*128+c]
            negD = []
            d_all = dpool.tile([128, MC * P], F32, tag="dps")
            for m in range(MC):
                d_ps = d_all[:, m * P:(m + 1) * P]
                for k in range(KC):
                    bq = B_dup[k][:].rearrange("c (n q) -> c n q", q=2)
                    nc.tensor.matmul(
                        d_ps,
                        lhsT=w2n_sb[k][:, m * 128:(m + 1) * 128],
                        rhs=bq[:, cs, 0:1],
                        start=(k == 0), stop=(k == KC - 1))
                nd = wpool.tile([128, P], BF16, tag=f"nd{m}", name=f"nd_{m}")
                nc.scalar.activation(
                    nd[:], d_ps, mybir.ActivationFunctionType.Identity,
                    bias=t2_sb[:, m:m + 1], scale=1.0)
                negD.append(nd)
            scene_negD[s] = negD

            # mx(s): Mx[c, i, j] = max(A''[c, j], B''[c, i]) fp16 at DVE 2x.
            # Scene 0 in i-quarters so the first L2 blocks start early.
            mx = [mxpool.tile([128, P * P], BF16, tag=f"mx{k}", name=f"mx_{k}")
                  for k in range(KC)]
            pieces = (((0, 16), (16, 64)) if s == 0 else ((0, 64),))
            for i0, i1 in pieces:
                ni = i1 - i0
                for k in range(KC):
                    nc.vector.tensor_tensor(
                        out=mx[k][:].rearrange("c (i w q) -> c i w q",
                                               w=P // 2, q=2)[:, i0:i1, :, :],
                        in0=A_bf[k][:, cs].rearrange("c (w q) -> c w q", q=2)
                            .unsqueeze(1).broadcast_to((128, ni, P // 2, 2)),
                        in1=B_dup[k][:, cs2].rearrange("c (i q) -> c i q", q=2)
                            [:, i0:i1, :].unsqueeze(2)
                            .broadcast_to((128, ni, P // 2, 2)),
                        op=mybir.AluOpType.max)

            # L2(s): layer-2 matmuls (fp16), ACT copies PSUM->SBUF fp16
            y2 = [ypool.tile([128, P * P], BF16, tag=f"y2{m}", name=f"y2_{m}")
                  for m in range(MC)]
            for blk in range(8):
                bs = slice(blk * 512, (blk + 1) * 512)
                for m in range(MC):
                    ps_t = mmpool.tile([128, 512], F32, tag="ps")
                    for k in range(KC):
                        nc.tensor.matmul(
                            ps_t[:],
                            lhsT=w2b_sb[k][:, m * 128:(m + 1) * 128],
                            rhs=mx[k][:, bs],
                            start=(k == 0), stop=(k == KC - 1))
                    nc.scalar.copy(y2[m][:, bs], ps_t[:])
            scene_y2[s] = y2

            # rest of phase-0 once scene 0's consumers are queued
            if s == 0:
                emit_phase0_piece(128, NLOC)
            # tree(s-1) after mx(s)+L2(s): DVE's tree work queues behind
            # mx(s), so PE(s) is never starved by tree latency.
            if s >= 1:
                emit_tree(s - 1, quarters=1)
        emit_tree(S_LOC - 1, quarters=4, piece_epi=True)

    nc.compile()
    return nc


def _prep_inputs(inputs):
    h = np.ascontiguousarray(inputs["h_states"], np.float32)
    pos = np.ascontiguousarray(inputs["last_pos"], np.float32)
    spd = np.ascontiguousarray(inputs["speed"], np.float32)
    W_emb = np.asarray(inputs["W_emb"], np.float32)
    b_emb = np.asarray(inputs["b_emb"], np.float32)
    W1 = np.asarray(inputs["W1"], np.float32)
    b1 = np.asarray(inputs["b1"], np.float32)
    g1 = np.asarray(inputs["g1"], np.float32)
    be1 = np.asarray(inputs["be1"], np.float32)
    m1 = np.asarray(inputs["m1"], np.float32)
    v1 = np.asarray(inputs["v1"], np.float32)
    W2 = np.asarray(inputs["W2"], np.float32)
    b2 = np.asarray(inputs["b2"], np.float32)
    g2 = np.asarray(inputs["g2"], np.float32)
    be2 = np.asarray(inputs["be2"], np.float32)
    m2 = np.asarray(inputs["m2"], np.float32)
    v2 = np.asarray(inputs["v2"], np.float32)

    s1 = g1 / np.sqrt(v1 + EPS)
    t1 = be1 - m1 * s1
    s2 = g2 / np.sqrt(v2 + EPS)
    t2 = be2 - m2 * s2 + b2 * s2
    R3 = W_emb @ W1[H:H + E, :]                       # [3, MID]
    W1aug = np.concatenate([W1[:H, :], R3], axis=0)   # [67, MID]
    c0v = b1 + b_emb @ W1[H:H + E, :]                 # [MID]
    ca = s1 * c0v + t1
    W2s = W2 * s2[None, :]                            # [MID, BOT]

    X = np.concatenate([h, pos[:, 0:1], pos[:, 1:2], spd], axis=1)  # [N, 67]

    W1aug16 = W1aug.astype(np.float16)                # [67, 512]
    mvcols = [np.stack([s1[k * 128:(k + 1) * 128], ca[k * 128:(k + 1) * 128]],
                       axis=1) for k in range(KC)]    # KC x [128, 2]
    t2cols = np.stack([t2[m * 128:(m + 1) * 128] for m in range(MC)], axis=1)
    mvt2 = np.concatenate(mvcols + [t2cols], axis=1)  # [128, 10]
    W2s16 = W2s.astype(np.float16)
    w2cols = [W2s16[k * 128:(k + 1) * 128, :] for k in range(KC)] + \
             [(-W2s16)[k * 128:(k + 1) * 128, :] for k in range(KC)]
    w2pack = np.concatenate(w2cols, axis=1)           # [128, 2048]

    common = dict(
        mvt2=np.ascontiguousarray(mvt2, np.float32),
        w2pack=np.ascontiguousarray(w2pack),
    )
    in_maps = []
    for c in range(NCORES):
        xc16 = X[c * NLOC:(c + 1) * NLOC, :].T.astype(np.float16)  # [67, 512]
        m = dict(common)
        m["big67"] = np.ascontiguousarray(
            np.concatenate([xc16, W1aug16], axis=1))               # [67, 1024]
        m["pw1b"] = np.ascontiguousarray(
            np.concatenate([xc16[H:H + 2, :],
                            R3[0:2, :].astype(np.float16)], axis=1))  # [2, 1024]
        in_maps.append(m)
    return in_maps


def kernel(**inputs):
    if "nc" not in _CACHE:
        _CACHE["nc"] = _build_program()
    nc = _CACHE["nc"]
    in_maps = _prep_inputs(inputs)
    res = run_bass_kernel_spmd(nc, in_maps, list(range(NCORES)))
    out = np.concatenate(
        [np.ascontiguousarray(res.results[c]["out"]).T for c in range(NCORES)],
        axis=0)
    return np.ascontiguousarray(out, np.float32)
